# revision 7
# baseline (speedup 1.0000x reference)
"""ContextualNeuronPool Trainium2 kernel (8-core SPMD).

Math (per token t, with K=8 selected pool entries p_k = idx[t,k], w = softmax(pattern_weights[t])):
    combined[t, f] = sum_k w_k * bp_eff[p_k, f]                  (base term, via routing matrix A)
                   + (sum_k w_k * (G[p_k] @ x[t])) @ adj_proj    (modulation term, via MoE grouping)
    out[t] = gelu(combined[t]) @ W2^T + w2_b
where G[p] = cm_w block [64, 1024] for pool entry p and bp_eff folds the cm_b bias:
    bp_eff = base_patterns + cm_b.reshape(P, M) @ adj_proj       (exact constant folding, host side)

Sharding:
  Phase A (expert-sharded): core c owns pool entries [64c, 64c+64). Host groups (token, k)
  pairs by pool entry. Per core, entries are sorted by token count into 64 slots; slot s is
  padded to a global slot_sizes[s] so all cores run an identical program. Host ships gathered
  x columns (bf16); modulation vectors come from per-slot matmul chains. Each slot's vectors
  are indirect-scattered into a pair table sorted by destination core (pad rows dropped via
  OOB), AllToAll'd (each pair goes only to the core owning its token), then gathered per
  token tile by index.
  Phase B (token-sharded): core c owns tokens [512c, 512c+512). Softmax, routing matrix A,
  base-term matmul A @ bp_eff (overlaps phase A DMA), + modulation @ adj_proj, gelu, @ W2^T.
"""

import numpy as np
import ml_dtypes

import concourse.bacc as bacc
import concourse.bass as bass
import concourse.tile as tile
import concourse.mybir as mybir
from concourse.bass_utils import run_bass_kernel_spmd
from concourse.masks import make_identity

BF16 = mybir.dt.bfloat16
F32 = mybir.dt.float32
I32 = mybir.dt.int32
AF = mybir.ActivationFunctionType
ALU = mybir.AluOpType

POOL, D, DFF, M = 512, 1024, 4096, 64
B, S, K = 2, 2048, 8
NCORES = 8
NTOK = B * S                  # 4096 tokens
T = NTOK // NCORES            # 512 tokens per core
EPC = POOL // NCORES          # 64 experts (pool entries) per core
DC = D // 128                 # 8 contraction chunks
TT = T // 128                 # 4 token tiles per core
PC = POOL // 128              # 4 pool chunks
FT = DFF // 128               # 32 d_ff tiles
GRP = 8                       # expert slots per DMA load group
NGRP = EPC // GRP             # 8 groups
W2C = 4                       # f-tiles per W2 chunk tile
PAD_IDX = 1 << 20             # scatter index for padding rows (OOB-dropped)


def _build_program(slot_sizes, Q):
    slot_off = np.concatenate([[0], np.cumsum(slot_sizes)]).astype(int)
    TW = int(slot_off[-1])          # packed xgt width
    NA2 = NCORES * Q                # pair table rows (src-chunked)

    nc = bacc.Bacc("TRN2", target_bir_lowering=False, debug=False, num_devices=NCORES)

    xgt_d = nc.dram_tensor("xgt", [128, DC, TW], BF16, kind="ExternalInput")
    cmt_d = nc.dram_tensor("cmt", [128, DC, EPC * M], BF16, kind="ExternalInput")
    bp_d = nc.dram_tensor("bp", [POOL, DFF], BF16, kind="ExternalInput")
    adj_d = nc.dram_tensor("adjp", [M, DFF], BF16, kind="ExternalInput")
    w2t_d = nc.dram_tensor("w2t", [128, FT, D], BF16, kind="ExternalInput")
    idx_d = nc.dram_tensor("idx2", [128, TT * K], I32, kind="ExternalInput")
    pw_d = nc.dram_tensor("pw2", [128, TT * K], F32, kind="ExternalInput")
    gidx_d = nc.dram_tensor("gidx", [128, TT * K], I32, kind="ExternalInput")
    scidx_d = nc.dram_tensor("scidx", [128, EPC], I32, kind="ExternalInput")
    out_d = nc.dram_tensor("out", [T, D], F32, kind="ExternalOutput")

    with tile.TileContext(nc) as tc:
        with tc.tile_pool(name="const", bufs=1) as const, \
             tc.tile_pool(name="small", bufs=1) as small, \
             tc.tile_pool(name="ab", bufs=2) as ab_pool, \
             tc.tile_pool(name="rg", bufs=8) as rg_pool, \
             tc.tile_pool(name="rw", bufs=2) as rw_pool, \
             tc.tile_pool(name="comb", bufs=3) as comb_pool, \
             tc.tile_pool(name="outp", bufs=2) as out_pool, \
             tc.tile_pool(name="dram", bufs=1, space="DRAM") as dram:

            # ---------------- constants / small inputs ----------------
            ident = const.tile([128, 128], BF16)
            make_identity(nc, ident[:])
            iota_f = const.tile([128, POOL], F32)
            nc.gpsimd.iota(iota_f[:], pattern=[[1, POOL]], base=0, channel_multiplier=0,
                           allow_small_or_imprecise_dtypes=True)

            idx_i = small.tile([128, TT, K], I32)
            pw_sb = small.tile([128, TT, K], F32)
            gidx_sb = small.tile([128, TT * K], I32)
            scidx_sb = small.tile([128, EPC], I32)
            nc.scalar.dma_start(out=scidx_sb[:], in_=scidx_d[:, :])
            nc.scalar.dma_start(out=idx_i[:], in_=idx_d[:, :])
            nc.scalar.dma_start(out=pw_sb[:], in_=pw_d[:, :])
            nc.scalar.dma_start(out=gidx_sb[:], in_=gidx_d[:, :])

            idxf = small.tile([128, TT, K], F32)
            w_sb = small.tile([128, TT, K], F32)
            negmax = small.tile([128, TT, 1], F32)
            sume = small.tile([128, TT, 1], F32)
            rec = small.tile([128, TT, 1], F32)
            for ti in range(TT):
                nc.vector.reduce_max(out=negmax[:, ti], in_=pw_sb[:, ti],
                                     axis=mybir.AxisListType.X, negate=True)
                nc.scalar.activation(out=w_sb[:, ti], in_=pw_sb[:, ti], func=AF.Exp,
                                     bias=negmax[:, ti], scale=1.0, accum_out=sume[:, ti])
                nc.vector.reciprocal(out=rec[:, ti], in_=sume[:, ti])
                nc.vector.tensor_scalar_mul(out=w_sb[:, ti], in0=w_sb[:, ti], scalar1=rec[:, ti])
            nc.vector.tensor_copy(out=idxf[:], in_=idx_i[:])

            pair_tab = dram.tile([NA2, M], BF16)
            at2_tab = dram.tile([NA2, M], BF16)

            at_tiles = []   # A^T chunk tiles [128 pool, T] bf16
            for pj in range(PC):
                at_tiles.append(const.tile([128, T], BF16, tag=f"at{pj}", name=f"at{pj}"))
            wmodT = const.tile([M, T], BF16, tag="wmodT")
            stage_tiles = []
            act_tiles = []
            for ft in range(FT):
                stage_tiles.append(const.tile([128, T], BF16, tag=f"stg{ft}", name=f"stg{ft}"))
                act_tiles.append(const.tile([128, T], BF16, tag=f"act{ft}", name=f"act{ft}"))
            adj_sb = const.tile([M, DFF], BF16, tag="adj")

            # helper: A build (DVE) for one token tile
            def a_build(ti):
                a_t = ab_pool.tile([128, POOL], BF16, tag="a", name=f"a{ti}")
                for k in range(K):
                    if k == 0:
                        nc.vector.tensor_scalar(out=a_t[:], in0=iota_f[:],
                                                scalar1=idxf[:, ti, k:k + 1],
                                                scalar2=w_sb[:, ti, k:k + 1],
                                                op0=ALU.is_equal, op1=ALU.mult)
                    else:
                        tmp = ab_pool.tile([128, POOL], BF16, tag="atmp")
                        nc.vector.tensor_scalar(out=tmp[:], in0=iota_f[:],
                                                scalar1=idxf[:, ti, k:k + 1],
                                                scalar2=w_sb[:, ti, k:k + 1],
                                                op0=ALU.is_equal, op1=ALU.mult)
                        nc.vector.tensor_tensor(out=a_t[:], in0=a_t[:], in1=tmp[:], op=ALU.add)
                return a_t

            with tc.tile_pool(name="psA", bufs=2, space="PSUM") as psA, \
                 tc.tile_pool(name="psT", bufs=2, space="PSUM") as psT, \
                 tc.tile_pool(name="psB", bufs=2, space="PSUM") as psB, \
                 tc.tile_pool(name="psW", bufs=1, space="PSUM") as psW, \
                 tc.tile_pool(name="xg", bufs=2) as xg_pool, \
                 tc.tile_pool(name="cm", bufs=2) as cm_pool, \
                 tc.tile_pool(name="pr", bufs=4) as pr_pool, \
                 tc.tile_pool(name="bpp", bufs=1) as bp_pool:

                bp_tiles = []
                for pj in range(PC):
                    t_ = bp_pool.tile([128, DFF], BF16, tag=f"bp{pj}", name=f"bp{pj}")
                    bp_tiles.append(t_)

                # sync queue: group loads g0-g3, bp, g4-g7, adj, w2
                xg_tiles, cm_tiles = {}, {}

                def load_group(g):
                    glo, ghi = int(slot_off[g * GRP]), int(slot_off[(g + 1) * GRP])
                    gw = ghi - glo
                    xg = xg_pool.tile([128, DC, gw], BF16, tag="xg", name=f"xg{g}")
                    cm = cm_pool.tile([128, DC, GRP * M], BF16, tag="cm", name=f"cm{g}")
                    nc.sync.dma_start(out=xg[:], in_=xgt_d[:, :, glo:ghi])
                    nc.sync.dma_start(
                        out=cm[:], in_=cmt_d[:, :, g * GRP * M:(g + 1) * GRP * M])
                    xg_tiles[g], cm_tiles[g] = xg, cm

                for g in range(4):
                    load_group(g)
                for pj in range(PC):
                    nc.sync.dma_start(out=bp_tiles[pj][:], in_=bp_d[pj * 128:(pj + 1) * 128, :])
                for g in range(4, NGRP):
                    load_group(g)
                nc.sync.dma_start(out=adj_sb[:], in_=adj_d[:, :])

                # phase A slots for one group (PE + DVE copies + gpsimd scatters)
                def slots_group(g):
                    glo = int(slot_off[g * GRP])
                    xg, cm = xg_tiles[g], cm_tiles[g]
                    for s in range(GRP):
                        sl = g * GRP + s
                        m_s = int(slot_sizes[sl])
                        lo = int(slot_off[sl] - glo)
                        ps = psA.tile([128, M], F32)
                        for j in range(DC):
                            nc.tensor.matmul(ps[:m_s, :],
                                             lhsT=xg[:, j, lo:lo + m_s],
                                             rhs=cm[:, j, s * M:(s + 1) * M],
                                             start=(j == 0), stop=(j == DC - 1))
                        pr = pr_pool.tile([128, M], BF16, tag="pr")
                        nc.vector.tensor_copy(out=pr[:m_s, :], in_=ps[:m_s, :])
                        nc.gpsimd.indirect_dma_start(
                            out=pair_tab[:],
                            out_offset=bass.IndirectOffsetOnAxis(
                                ap=scidx_sb[:, sl:sl + 1], axis=0),
                            in_=pr[:], in_offset=None,
                            bounds_check=NA2 - 1, oob_is_err=False)

                # A^T transposes for one token tile (PE + scalar copies)
                def at_transpose(ti, a_t):
                    for pj in range(PC):
                        pst = psT.tile([128, 128], BF16)
                        nc.tensor.transpose(pst[:], a_t[:, pj * 128:(pj + 1) * 128], ident[:])
                        nc.scalar.activation(out=at_tiles[pj][:, ti * 128:(ti + 1) * 128],
                                             in_=pst[:], func=AF.Copy)

                # interleaved emission: phase A groups + A build + transposes + base
                for ti in range(TT):
                    a_t = a_build(ti)               # DVE
                    slots_group(ti)                 # PE g0..g3 + DVE pr copies
                    at_transpose(ti, a_t)           # PE (after slots of this group)
                slots_group(4)
                slots_group(5)
                # base term: combined^T staged = (A @ bp_eff)^T  (PE during g6/g7 DMA)
                for ft in range(FT):
                    psb = psB.tile([128, T], F32)
                    for pj in range(PC):
                        nc.tensor.matmul(psb[:], lhsT=bp_tiles[pj][:, ft * 128:(ft + 1) * 128],
                                         rhs=at_tiles[pj][:], start=(pj == 0), stop=(pj == PC - 1))
                    nc.scalar.activation(out=stage_tiles[ft][:], in_=psb[:], func=AF.Copy)
                slots_group(6)
                slots_group(7)

                nc.gpsimd.collective_compute(
                    "AllToAll", ALU.bypass,
                    replica_groups=[list(range(NCORES))],
                    ins=[pair_tab[:].opt()], outs=[at2_tab[:].opt()],
                )

                # gather pair vectors; weighted k-sum via scalar scale + PE transpose-acc
                for ti in range(TT):
                    rgs = []
                    for k in range(K):
                        rgt = rg_pool.tile([128, M], BF16, tag="rg", name=f"rg{ti}_{k}")
                        nc.gpsimd.indirect_dma_start(
                            out=rgt[:], out_offset=None,
                            in_=at2_tab[:],
                            in_offset=bass.IndirectOffsetOnAxis(
                                ap=gidx_sb[:, ti * K + k: ti * K + k + 1], axis=0),
                        )
                        rgs.append(rgt)
                    rw = rw_pool.tile([128, K, M], BF16, tag="rw")
                    for k in range(K):
                        nc.scalar.activation(out=rw[:, k], in_=rgs[k][:], func=AF.Copy,
                                             scale=w_sb[:, ti, k:k + 1])
                    psw = psW.tile([M, 128], F32)
                    for k in range(K):
                        nc.tensor.matmul(psw[:], lhsT=rw[:, k], rhs=ident[:],
                                         start=(k == 0), stop=(k == K - 1))
                    nc.vector.tensor_copy(out=wmodT[:, ti * 128:(ti + 1) * 128], in_=psw[:])

            # ---------------- pass 2: adj term + gelu, then W2 ----------------
            # w2 tiles allocated after the front pools close: their SBUF space
            # reuses xg/cm/bp, so the loads start once phase A frees it.
            with tc.tile_pool(name="w2p", bufs=1) as w2_pool:
                w2_tiles = []
                for j in range(FT // W2C):
                    w2_tiles.append(w2_pool.tile([128, W2C, D], BF16, tag=f"w2_{j}",
                                                 name=f"w2_{j}"))
                    nc.sync.dma_start(out=w2_tiles[j][:],
                                      in_=w2t_d[:, j * W2C:(j + 1) * W2C, :])

                def w2_half(psO, trange):
                    for fc in range(FT):
                        w2c = w2_tiles[fc // W2C]
                        for t in trange:
                            for dd in range(2):
                                nc.tensor.matmul(psO[(t, dd)][:],
                                                 lhsT=act_tiles[fc][:, t * 128:(t + 1) * 128],
                                                 rhs=w2c[:, fc % W2C, dd * 512:(dd + 1) * 512],
                                                 start=(fc == 0), stop=(fc == FT - 1))

                def drain_half(psO, trange):
                    for t in trange:
                        for dd in range(2):
                            ob = out_pool.tile([128, 512], F32)
                            nc.vector.tensor_copy(out=ob[:], in_=psO[(t, dd)][:])
                            nc.sync.dma_start(
                                out=out_d[t * 128:(t + 1) * 128, dd * 512:(dd + 1) * 512],
                                in_=ob[:])

                with tc.tile_pool(name="psOa", bufs=1, space="PSUM") as psOa_pool:
                    psOa = {}
                    for t in (0, 1):
                        for dd in range(2):
                            psOa[(t, dd)] = psOa_pool.tile([128, 512], F32, tag=f"oa{t}_{dd}",
                                                           name=f"opsa{t}_{dd}")
                    with tc.tile_pool(name="psC", bufs=2, space="PSUM") as psC:
                        for ft in range(FT):
                            psc = psC.tile([128, T], F32, tag="psc", name=f"psc{ft}")
                            nc.tensor.matmul(psc[:], lhsT=adj_sb[:, ft * 128:(ft + 1) * 128],
                                             rhs=wmodT[:], start=True, stop=True)
                            comb = comb_pool.tile([128, T], BF16, tag="comb", name=f"comb{ft}")
                            nc.vector.tensor_tensor(out=comb[:], in0=stage_tiles[ft][:],
                                                    in1=psc[:], op=ALU.add)
                            nc.scalar.activation(out=act_tiles[ft][:], in_=comb[:], func=AF.Gelu)
                        w2_half(psOa, (0, 1))
                    with tc.tile_pool(name="psOb", bufs=1, space="PSUM") as psOb_pool:
                        psOb = {}
                        for t in (2, 3):
                            for dd in range(2):
                                psOb[(t, dd)] = psOb_pool.tile([128, 512], F32, tag=f"ob{t}_{dd}",
                                                               name=f"opsb{t}_{dd}")
                        w2_half(psOb, (2, 3))
                        drain_half(psOa, (0, 1))
                        drain_half(psOb, (2, 3))

    nc.compile()
    return nc


def _routing(idx):
    """Group (t, k) pairs by pool entry; build slot packing + AllToAll chunk layout."""
    flat_e = idx.ravel()
    order = np.argsort(flat_e, kind="stable")  # pairs sorted by (expert, t, k)
    counts = np.bincount(flat_e, minlength=POOL)
    starts = np.zeros(POOL, dtype=np.int64)
    starts[1:] = np.cumsum(counts)[:-1]
    tok_sorted = (np.arange(NTOK * K, dtype=np.int64) // K)[order]

    # per core: experts sorted by count desc -> slots
    slot_expert = np.zeros((NCORES, EPC), dtype=np.int64)
    for c in range(NCORES):
        cnt = counts[c * EPC:(c + 1) * EPC]
        slot_expert[c] = c * EPC + np.argsort(-cnt, kind="stable")
    slot_counts = counts[slot_expert]                       # [NCORES, EPC]
    slot_sizes = ((slot_counts.max(axis=0) + 15) // 16 * 16).astype(np.int64)
    slot_sizes = np.maximum(slot_sizes, 16)
    assert slot_sizes.max() <= 128, f"slot overflow {slot_sizes.max()}"
    slot_off = np.concatenate([[0], np.cumsum(slot_sizes)])

    # AllToAll chunk sizes: pairs per (src expert-core, dst token-core)
    src_of_pair = flat_e // EPC
    dst_of_pair = (np.arange(NTOK * K, dtype=np.int64) // K) // T
    cnt_cd = np.bincount(src_of_pair * NCORES + dst_of_pair,
                         minlength=NCORES * NCORES).reshape(NCORES, NCORES)
    Q = int((cnt_cd.max() + 15) // 16 * 16)

    # per-pair table row (in src core's pair_tab, dst-chunked) + receiver row
    scidx = np.full((NCORES, 128, EPC), PAD_IDX, dtype=np.int64)
    grow = np.empty(NTOK * K, dtype=np.int64)   # row in receiver's at2 = src*Q + rank
    for c in range(NCORES):
        rank = np.zeros(NCORES, dtype=np.int64)
        for s in range(EPC):
            e = int(slot_expert[c, s])
            seg = order[starts[e]:starts[e] + counts[e]]   # pair flat ids, token-sorted
            toks = tok_sorted[starts[e]:starts[e] + counts[e]]
            for i in range(len(seg)):
                d = int(toks[i]) // T
                r = rank[d]; rank[d] += 1
                scidx[c, i, s] = d * Q + r
                grow[seg[i]] = c * Q + r
        assert rank.max() <= Q
    grow = grow.reshape(NTOK, K)
    return (order, counts, starts, tok_sorted, slot_expert, slot_sizes, slot_off,
            int(slot_off[-1]), Q, scidx, grow)


def _tile128(a):
    """[T, X] -> [128, TT*X] with row p holding token ti*128+p at cols ti*X+..."""
    X = a.shape[1]
    return np.ascontiguousarray(
        a.reshape(TT, 128, X).transpose(1, 0, 2).reshape(128, TT * X))


def _prepare_inputs(x, selected_indices, pattern_weights, base_patterns, cm_w, cm_b,
                    adj_proj, w2_w):
    bf = ml_dtypes.bfloat16
    x2 = np.ascontiguousarray(x.reshape(NTOK, D), dtype=np.float32)
    idx = np.ascontiguousarray(selected_indices.reshape(NTOK, K)).astype(np.int32)
    pw = np.ascontiguousarray(pattern_weights.reshape(NTOK, K), dtype=np.float32)

    # exact constant folding of the cm_b bias into the base patterns
    bp_eff = base_patterns.astype(np.float32) + cm_b.reshape(POOL, M).astype(np.float32) @ adj_proj.astype(np.float32)
    bp_bf = bp_eff.astype(bf)
    adj_bf = adj_proj.astype(bf)
    w2t = np.ascontiguousarray(w2_w.T).astype(bf)               # [DFF, D]
    w2th = np.ascontiguousarray(
        w2t.reshape(FT, 128, D).transpose(1, 0, 2))             # [128, FT, D]
    x2t_bf = np.ascontiguousarray(x2.T).astype(bf)              # [D, NTOK]

    (order, counts, starts, tok_sorted, slot_expert, slot_sizes, slot_off, TW,
     Q, scidx, grow) = _routing(idx)

    cm3 = cm_w.reshape(POOL, M, D)
    in_maps = []
    for c in range(NCORES):
        xgt = np.zeros((D, TW), dtype=bf)
        cmt = np.empty((D, EPC * M), dtype=bf)
        for s in range(EPC):
            e = int(slot_expert[c, s])
            seg = tok_sorted[starts[e]:starts[e] + counts[e]]
            off = int(slot_off[s])
            xgt[:, off:off + len(seg)] = x2t_bf[:, seg]
            cmt[:, s * M:(s + 1) * M] = cm3[e].T.astype(bf)
        xgt128 = np.ascontiguousarray(xgt.reshape(DC, 128, TW).transpose(1, 0, 2))
        cmt128 = np.ascontiguousarray(cmt.reshape(DC, 128, EPC * M).transpose(1, 0, 2))
        in_maps.append({
            "xgt": xgt128,
            "cmt": cmt128,
            "bp": bp_bf,
            "adjp": adj_bf,
            "w2t": w2th,
            "idx2": _tile128(idx[c * T:(c + 1) * T]),
            "pw2": _tile128(pw[c * T:(c + 1) * T]),
            "gidx": _tile128(grow[c * T:(c + 1) * T]).astype(np.int32),
            "scidx": np.ascontiguousarray(scidx[c]).astype(np.int32),
        })
    return in_maps, (slot_sizes, Q)


def _run(inputs, trace=False):
    in_maps, (slot_sizes, Q) = _prepare_inputs(
        inputs["x"], inputs["selected_indices"], inputs["pattern_weights"],
        inputs["base_patterns"], inputs["cm_w"], inputs["cm_b"],
        inputs["adj_proj"], inputs["w2_w"])
    nc = _build_program(slot_sizes, Q)
    res = run_bass_kernel_spmd(nc, in_maps, core_ids=list(range(NCORES)), trace=trace)
    out = np.concatenate([res.results[c]["out"] for c in range(NCORES)], axis=0)
    out = out + np.asarray(inputs["w2_b"], dtype=np.float32)[None, :]
    return out.reshape(B, S, D).astype(np.float32), res


def kernel(**inputs) -> np.ndarray:
    out, _ = _run(inputs, trace=False)
    return out


# revision 11
# speedup vs baseline: 1.0230x; 1.0230x over previous
"""ContextualNeuronPool Trainium2 kernel (8-core SPMD).

Math (per token t, with K=8 selected pool entries p_k = idx[t,k], w = softmax(pattern_weights[t])):
    combined[t, f] = sum_k w_k * bp_eff[p_k, f]                  (base term, via routing matrix A)
                   + (sum_k w_k * (G[p_k] @ x[t])) @ adj_proj    (modulation term, via MoE grouping)
    out[t] = gelu(combined[t]) @ W2^T + w2_b
where G[p] = cm_w block [64, 1024] for pool entry p and bp_eff folds the cm_b bias:
    bp_eff = base_patterns + cm_b.reshape(P, M) @ adj_proj       (exact constant folding, host side)

Sharding:
  Phase A (expert-sharded): core c owns pool entries [64c, 64c+64). Host groups (token, k)
  pairs by pool entry. Per core, entries are sorted by token count into 64 slots; slot s is
  padded to a global slot_sizes[s] so all cores run an identical program. Host ships gathered
  x columns (bf16); modulation vectors come from per-slot matmul chains. Each slot's vectors
  are indirect-scattered into a pair table sorted by destination core (pad rows dropped via
  OOB), AllToAll'd (each pair goes only to the core owning its token), then gathered per
  token tile by index.
  Phase B (token-sharded): core c owns tokens [512c, 512c+512). Softmax, routing matrix A,
  base-term matmul A @ bp_eff (overlaps phase A DMA), + modulation @ adj_proj, gelu, @ W2^T.
"""

import numpy as np
import ml_dtypes

import concourse.bacc as bacc
import concourse.bass as bass
import concourse.tile as tile
import concourse.mybir as mybir
from concourse.bass_utils import run_bass_kernel_spmd
from concourse.masks import make_identity

BF16 = mybir.dt.bfloat16
F32 = mybir.dt.float32
I32 = mybir.dt.int32
AF = mybir.ActivationFunctionType
ALU = mybir.AluOpType

POOL, D, DFF, M = 512, 1024, 4096, 64
B, S, K = 2, 2048, 8
NCORES = 8
NTOK = B * S                  # 4096 tokens
T = NTOK // NCORES            # 512 tokens per core
EPC = POOL // NCORES          # 64 experts (pool entries) per core
DC = D // 128                 # 8 contraction chunks
TT = T // 128                 # 4 token tiles per core
PC = POOL // 128              # 4 pool chunks
FT = DFF // 128               # 32 d_ff tiles
GRP = 8                       # expert slots per DMA load group
NGRP = EPC // GRP             # 8 groups
W2C = 4                       # f-tiles per W2 chunk tile
PAD_IDX = 1 << 20             # scatter index for padding rows (OOB-dropped)


def _build_program(slot_sizes, Q):
    slot_off = np.concatenate([[0], np.cumsum(slot_sizes)]).astype(int)
    TW = int(slot_off[-1])          # packed xgt width
    NA2 = NCORES * Q                # pair table rows (src-chunked)

    nc = bacc.Bacc("TRN2", target_bir_lowering=False, debug=False, num_devices=NCORES)

    xgt_d = nc.dram_tensor("xgt", [128, DC, TW], BF16, kind="ExternalInput")
    cmt_d = nc.dram_tensor("cmt", [128, DC, EPC * M], BF16, kind="ExternalInput")
    bp_d = nc.dram_tensor("bp", [POOL, DFF], BF16, kind="ExternalInput")
    adj_d = nc.dram_tensor("adjp", [M, DFF], BF16, kind="ExternalInput")
    w2t_d = nc.dram_tensor("w2t", [128, FT, D], BF16, kind="ExternalInput")
    idx_d = nc.dram_tensor("idx2", [128, TT * K], I32, kind="ExternalInput")
    pw_d = nc.dram_tensor("pw2", [128, TT * K], F32, kind="ExternalInput")
    gidx_d = nc.dram_tensor("gidx", [128, TT * K], I32, kind="ExternalInput")
    scidx_d = nc.dram_tensor("scidx", [128, EPC], I32, kind="ExternalInput")
    out_d = nc.dram_tensor("out", [T, D], F32, kind="ExternalOutput")

    with tile.TileContext(nc) as tc:
        with tc.tile_pool(name="const", bufs=1) as const, \
             tc.tile_pool(name="small", bufs=1) as small, \
             tc.tile_pool(name="ab", bufs=2) as ab_pool, \
             tc.tile_pool(name="rg", bufs=8) as rg_pool, \
             tc.tile_pool(name="rw", bufs=2) as rw_pool, \
             tc.tile_pool(name="comb", bufs=3) as comb_pool, \
             tc.tile_pool(name="outp", bufs=2) as out_pool, \
             tc.tile_pool(name="dram", bufs=1, space="DRAM") as dram:

            # ---------------- constants / small inputs ----------------
            ident = const.tile([128, 128], BF16)
            make_identity(nc, ident[:])
            iota_f = const.tile([128, POOL], F32)
            nc.gpsimd.iota(iota_f[:], pattern=[[1, POOL]], base=0, channel_multiplier=0,
                           allow_small_or_imprecise_dtypes=True)

            idx_i = small.tile([128, TT, K], I32)
            pw_sb = small.tile([128, TT, K], F32)
            gidx_sb = small.tile([128, TT * K], I32)
            scidx_sb = small.tile([128, EPC], I32)
            nc.scalar.dma_start(out=scidx_sb[:], in_=scidx_d[:, :])
            nc.scalar.dma_start(out=idx_i[:], in_=idx_d[:, :])
            nc.scalar.dma_start(out=pw_sb[:], in_=pw_d[:, :])
            nc.scalar.dma_start(out=gidx_sb[:], in_=gidx_d[:, :])

            idxf = small.tile([128, TT, K], F32)
            w_sb = small.tile([128, TT, K], F32)
            negmax = small.tile([128, TT, 1], F32)
            sume = small.tile([128, TT, 1], F32)
            rec = small.tile([128, TT, 1], F32)
            for ti in range(TT):
                nc.vector.reduce_max(out=negmax[:, ti], in_=pw_sb[:, ti],
                                     axis=mybir.AxisListType.X, negate=True)
                nc.scalar.activation(out=w_sb[:, ti], in_=pw_sb[:, ti], func=AF.Exp,
                                     bias=negmax[:, ti], scale=1.0, accum_out=sume[:, ti])
                nc.vector.reciprocal(out=rec[:, ti], in_=sume[:, ti])
                nc.vector.tensor_scalar_mul(out=w_sb[:, ti], in0=w_sb[:, ti], scalar1=rec[:, ti])
            nc.vector.tensor_copy(out=idxf[:], in_=idx_i[:])

            pair_tab = dram.tile([NA2, M], BF16)
            at2_tab = dram.tile([NA2, M], BF16)

            at_tiles = []   # A^T chunk tiles [128 pool, T] bf16
            for pj in range(PC):
                at_tiles.append(const.tile([128, T], BF16, tag=f"at{pj}", name=f"at{pj}"))
            wmodT = const.tile([M, T], BF16, tag="wmodT")
            stage_tiles = []
            act_tiles = []
            for ft in range(FT):
                stage_tiles.append(const.tile([128, T], BF16, tag=f"stg{ft}", name=f"stg{ft}"))
                act_tiles.append(const.tile([128, T], BF16, tag=f"act{ft}", name=f"act{ft}"))
            adj_sb = const.tile([M, DFF], BF16, tag="adj")

            # helper: A build (DVE) for one token tile
            def a_build(ti):
                a_t = ab_pool.tile([128, POOL], BF16, tag="a", name=f"a{ti}")
                for k in range(K):
                    if k == 0:
                        nc.vector.tensor_scalar(out=a_t[:], in0=iota_f[:],
                                                scalar1=idxf[:, ti, k:k + 1],
                                                scalar2=w_sb[:, ti, k:k + 1],
                                                op0=ALU.is_equal, op1=ALU.mult)
                    else:
                        tmp = ab_pool.tile([128, POOL], BF16, tag="atmp")
                        nc.vector.tensor_scalar(out=tmp[:], in0=iota_f[:],
                                                scalar1=idxf[:, ti, k:k + 1],
                                                scalar2=w_sb[:, ti, k:k + 1],
                                                op0=ALU.is_equal, op1=ALU.mult)
                        nc.vector.tensor_tensor(out=a_t[:], in0=a_t[:], in1=tmp[:], op=ALU.add)
                return a_t

            with tc.tile_pool(name="psA", bufs=4, space="PSUM") as psA, \
                 tc.tile_pool(name="psT", bufs=2, space="PSUM") as psT, \
                 tc.tile_pool(name="psB", bufs=2, space="PSUM") as psB, \
                 tc.tile_pool(name="xg", bufs=3) as xg_pool, \
                 tc.tile_pool(name="cm", bufs=3) as cm_pool, \
                 tc.tile_pool(name="pr", bufs=16) as pr_pool, \
                 tc.tile_pool(name="bpp", bufs=1) as bp_pool:

                bp_tiles = []
                for pj in range(PC):
                    t_ = bp_pool.tile([128, DFF], BF16, tag=f"bp{pj}", name=f"bp{pj}")
                    bp_tiles.append(t_)

                # sync queue: group loads g0-g3, bp, g4-g7, adj, w2
                xg_tiles, cm_tiles = {}, {}

                def load_group(g):
                    glo, ghi = int(slot_off[g * GRP]), int(slot_off[(g + 1) * GRP])
                    gw = ghi - glo
                    xg = xg_pool.tile([128, DC, gw], BF16, tag="xg", name=f"xg{g}")
                    cm = cm_pool.tile([128, DC, GRP * M], BF16, tag="cm", name=f"cm{g}")
                    nc.sync.dma_start(out=xg[:], in_=xgt_d[:, :, glo:ghi])
                    nc.sync.dma_start(
                        out=cm[:], in_=cmt_d[:, :, g * GRP * M:(g + 1) * GRP * M])
                    xg_tiles[g], cm_tiles[g] = xg, cm

                for g in range(3):
                    load_group(g)
                for pj in range(PC):
                    nc.sync.dma_start(out=bp_tiles[pj][:], in_=bp_d[pj * 128:(pj + 1) * 128, :])
                for g in range(3, NGRP):
                    load_group(g)
                nc.sync.dma_start(out=adj_sb[:], in_=adj_d[:, :])

                # phase A slots for one group (PE + DVE copies + gpsimd scatters)
                def slots_group(g):
                    glo = int(slot_off[g * GRP])
                    xg, cm = xg_tiles[g], cm_tiles[g]
                    for s in range(GRP):
                        sl = g * GRP + s
                        m_s = int(slot_sizes[sl])
                        lo = int(slot_off[sl] - glo)
                        ps = psA.tile([128, M], F32)
                        for j in range(DC):
                            nc.tensor.matmul(ps[:m_s, :],
                                             lhsT=xg[:, j, lo:lo + m_s],
                                             rhs=cm[:, j, s * M:(s + 1) * M],
                                             start=(j == 0), stop=(j == DC - 1))
                        pr = pr_pool.tile([128, M], BF16, tag="pr")
                        nc.vector.tensor_copy(out=pr[:m_s, :], in_=ps[:m_s, :])
                        nc.gpsimd.indirect_dma_start(
                            out=pair_tab[:],
                            out_offset=bass.IndirectOffsetOnAxis(
                                ap=scidx_sb[:, sl:sl + 1], axis=0),
                            in_=pr[:], in_offset=None,
                            bounds_check=NA2 - 1, oob_is_err=False)

                # A^T transposes for one token tile (PE + scalar copies)
                def at_transpose(ti, a_t):
                    for pj in range(PC):
                        pst = psT.tile([128, 128], BF16)
                        nc.tensor.transpose(pst[:], a_t[:, pj * 128:(pj + 1) * 128], ident[:])
                        nc.scalar.activation(out=at_tiles[pj][:, ti * 128:(ti + 1) * 128],
                                             in_=pst[:], func=AF.Copy)

                # base term: combined^T staged = (A @ bp_eff)^T
                def base_chains(fts):
                    for ft in fts:
                        psb = psB.tile([128, T], F32)
                        for pj in range(PC):
                            nc.tensor.matmul(psb[:],
                                             lhsT=bp_tiles[pj][:, ft * 128:(ft + 1) * 128],
                                             rhs=at_tiles[pj][:],
                                             start=(pj == 0), stop=(pj == PC - 1))
                        nc.scalar.activation(out=stage_tiles[ft][:], in_=psb[:], func=AF.Copy)

                # interleaved emission: phase A groups + A build + transposes + base.
                # base 16-31 sit after the last slot group so the PE stays busy
                # through the AllToAll window.
                for ti in range(TT):
                    a_t = a_build(ti)               # DVE
                    slots_group(ti)                 # PE g0..g3 + DVE pr copies
                    at_transpose(ti, a_t)           # PE (after slots of this group)
                slots_group(4)
                slots_group(5)
                slots_group(6)
                base_chains(range(0, 16))
                slots_group(7)
                base_chains(range(16, FT))

                nc.gpsimd.collective_compute(
                    "AllToAll", ALU.bypass,
                    replica_groups=[list(range(NCORES))],
                    ins=[pair_tab[:].opt()], outs=[at2_tab[:].opt()],
                )

                # all 32 gathers issued up front on the gpsimd queue (serialized
                # ~1.1us each); consumers pipeline per token tile.
                rg_tiles = {}
                for ti in range(TT):
                    for k in range(K):
                        rgt = rg_pool.tile([128, M], BF16, tag="rg", name=f"rg{ti}_{k}")
                        nc.gpsimd.indirect_dma_start(
                            out=rgt[:], out_offset=None,
                            in_=at2_tab[:],
                            in_offset=bass.IndirectOffsetOnAxis(
                                ap=gidx_sb[:, ti * K + k: ti * K + k + 1], axis=0),
                        )
                        rg_tiles[(ti, k)] = rgt

            # ---------------- pass 2: token-half pipelined tail ----------------
            # weighted k-sum via scalar scale + PE transpose-accumulate
            with tc.tile_pool(name="psW", bufs=1, space="PSUM") as psW, \
                 tc.tile_pool(name="w2p", bufs=1) as w2_pool:

                def wmod_ti(ti):
                    rw = rw_pool.tile([128, K, M], BF16, tag="rw")
                    for k in range(K):
                        nc.scalar.activation(out=rw[:, k], in_=rg_tiles[(ti, k)][:],
                                             func=AF.Copy, scale=w_sb[:, ti, k:k + 1])
                    psw = psW.tile([M, 128], F32)
                    for k in range(K):
                        nc.tensor.matmul(psw[:], lhsT=rw[:, k], rhs=ident[:],
                                         start=(k == 0), stop=(k == K - 1))
                    nc.vector.tensor_copy(out=wmodT[:, ti * 128:(ti + 1) * 128], in_=psw[:])

                w2_tiles = []
                for j in range(FT // W2C):
                    w2_tiles.append(w2_pool.tile([128, W2C, D], BF16, tag=f"w2_{j}",
                                                 name=f"w2_{j}"))
                    nc.sync.dma_start(out=w2_tiles[j][:],
                                      in_=w2t_d[:, j * W2C:(j + 1) * W2C, :])

                def adj_half(h, psC):
                    sl = slice(h * 256, (h + 1) * 256)
                    for ft in range(FT):
                        psc = psC.tile([128, 256], F32, tag="psc", name=f"psc{h}_{ft}")
                        nc.tensor.matmul(psc[:], lhsT=adj_sb[:, ft * 128:(ft + 1) * 128],
                                         rhs=wmodT[:, sl], start=True, stop=True)
                        comb = comb_pool.tile([128, 256], BF16, tag="comb",
                                              name=f"comb{h}_{ft}")
                        nc.vector.tensor_tensor(out=comb[:], in0=stage_tiles[ft][:, sl],
                                                in1=psc[:], op=ALU.add)
                        nc.scalar.activation(out=act_tiles[ft][:, sl], in_=comb[:],
                                             func=AF.Gelu)

                def w2_half(psO, trange):
                    for fc in range(FT):
                        w2c = w2_tiles[fc // W2C]
                        for t in trange:
                            for dd in range(2):
                                nc.tensor.matmul(psO[(t, dd)][:],
                                                 lhsT=act_tiles[fc][:, t * 128:(t + 1) * 128],
                                                 rhs=w2c[:, fc % W2C, dd * 512:(dd + 1) * 512],
                                                 start=(fc == 0), stop=(fc == FT - 1))

                def drain_half(psO, trange):
                    for t in trange:
                        for dd in range(2):
                            ob = out_pool.tile([128, 512], F32)
                            nc.vector.tensor_copy(out=ob[:], in_=psO[(t, dd)][:])
                            nc.sync.dma_start(
                                out=out_d[t * 128:(t + 1) * 128, dd * 512:(dd + 1) * 512],
                                in_=ob[:])

                with tc.tile_pool(name="psO", bufs=1, space="PSUM") as psO_pool, \
                     tc.tile_pool(name="psC", bufs=2, space="PSUM") as psC:
                    def alloc_psO(trange, nm):
                        return {(t, dd): psO_pool.tile([128, 512], F32, tag=f"o{t % 2}_{dd}",
                                                       name=f"o{nm}{t}_{dd}")
                                for t in trange for dd in range(2)}
                    wmod_ti(0)
                    wmod_ti(1)
                    adj_half(0, psC)
                    psOa = alloc_psO((0, 1), "a")
                    w2_half(psOa, (0, 1))
                    wmod_ti(2)
                    wmod_ti(3)
                    adj_half(1, psC)
                    drain_half(psOa, (0, 1))
                    psOb = alloc_psO((2, 3), "b")
                    w2_half(psOb, (2, 3))
                    drain_half(psOb, (2, 3))

    nc.compile()
    return nc


def _routing(idx):
    """Group (t, k) pairs by pool entry; build slot packing + AllToAll chunk layout."""
    flat_e = idx.ravel()
    order = np.argsort(flat_e, kind="stable")  # pairs sorted by (expert, t, k)
    counts = np.bincount(flat_e, minlength=POOL)
    starts = np.zeros(POOL, dtype=np.int64)
    starts[1:] = np.cumsum(counts)[:-1]
    tok_sorted = (np.arange(NTOK * K, dtype=np.int64) // K)[order]

    # per core: experts sorted by count desc -> slots
    slot_expert = np.zeros((NCORES, EPC), dtype=np.int64)
    for c in range(NCORES):
        cnt = counts[c * EPC:(c + 1) * EPC]
        slot_expert[c] = c * EPC + np.argsort(-cnt, kind="stable")
    slot_counts = counts[slot_expert]                       # [NCORES, EPC]
    slot_sizes = ((slot_counts.max(axis=0) + 15) // 16 * 16).astype(np.int64)
    slot_sizes = np.maximum(slot_sizes, 16)
    assert slot_sizes.max() <= 128, f"slot overflow {slot_sizes.max()}"
    slot_off = np.concatenate([[0], np.cumsum(slot_sizes)])

    # AllToAll chunk sizes: pairs per (src expert-core, dst token-core)
    src_of_pair = flat_e // EPC
    dst_of_pair = (np.arange(NTOK * K, dtype=np.int64) // K) // T
    cnt_cd = np.bincount(src_of_pair * NCORES + dst_of_pair,
                         minlength=NCORES * NCORES).reshape(NCORES, NCORES)
    Q = int((cnt_cd.max() + 15) // 16 * 16)

    # per-pair table row (in src core's pair_tab, dst-chunked) + receiver row
    scidx = np.full((NCORES, 128, EPC), PAD_IDX, dtype=np.int64)
    grow = np.empty(NTOK * K, dtype=np.int64)   # row in receiver's at2 = src*Q + rank
    for c in range(NCORES):
        rank = np.zeros(NCORES, dtype=np.int64)
        for s in range(EPC):
            e = int(slot_expert[c, s])
            seg = order[starts[e]:starts[e] + counts[e]]   # pair flat ids, token-sorted
            toks = tok_sorted[starts[e]:starts[e] + counts[e]]
            for i in range(len(seg)):
                d = int(toks[i]) // T
                r = rank[d]; rank[d] += 1
                scidx[c, i, s] = d * Q + r
                grow[seg[i]] = c * Q + r
        assert rank.max() <= Q
    grow = grow.reshape(NTOK, K)
    return (order, counts, starts, tok_sorted, slot_expert, slot_sizes, slot_off,
            int(slot_off[-1]), Q, scidx, grow)


def _tile128(a):
    """[T, X] -> [128, TT*X] with row p holding token ti*128+p at cols ti*X+..."""
    X = a.shape[1]
    return np.ascontiguousarray(
        a.reshape(TT, 128, X).transpose(1, 0, 2).reshape(128, TT * X))


def _prepare_inputs(x, selected_indices, pattern_weights, base_patterns, cm_w, cm_b,
                    adj_proj, w2_w):
    bf = ml_dtypes.bfloat16
    x2 = np.ascontiguousarray(x.reshape(NTOK, D), dtype=np.float32)
    idx = np.ascontiguousarray(selected_indices.reshape(NTOK, K)).astype(np.int32)
    pw = np.ascontiguousarray(pattern_weights.reshape(NTOK, K), dtype=np.float32)

    # exact constant folding of the cm_b bias into the base patterns
    bp_eff = base_patterns.astype(np.float32) + cm_b.reshape(POOL, M).astype(np.float32) @ adj_proj.astype(np.float32)
    bp_bf = bp_eff.astype(bf)
    adj_bf = adj_proj.astype(bf)
    w2t = np.ascontiguousarray(w2_w.T).astype(bf)               # [DFF, D]
    w2th = np.ascontiguousarray(
        w2t.reshape(FT, 128, D).transpose(1, 0, 2))             # [128, FT, D]
    x2t_bf = np.ascontiguousarray(x2.T).astype(bf)              # [D, NTOK]

    (order, counts, starts, tok_sorted, slot_expert, slot_sizes, slot_off, TW,
     Q, scidx, grow) = _routing(idx)

    cm3 = cm_w.reshape(POOL, M, D)
    in_maps = []
    for c in range(NCORES):
        xgt = np.zeros((D, TW), dtype=bf)
        cmt = np.empty((D, EPC * M), dtype=bf)
        for s in range(EPC):
            e = int(slot_expert[c, s])
            seg = tok_sorted[starts[e]:starts[e] + counts[e]]
            off = int(slot_off[s])
            xgt[:, off:off + len(seg)] = x2t_bf[:, seg]
            cmt[:, s * M:(s + 1) * M] = cm3[e].T.astype(bf)
        xgt128 = np.ascontiguousarray(xgt.reshape(DC, 128, TW).transpose(1, 0, 2))
        cmt128 = np.ascontiguousarray(cmt.reshape(DC, 128, EPC * M).transpose(1, 0, 2))
        in_maps.append({
            "xgt": xgt128,
            "cmt": cmt128,
            "bp": bp_bf,
            "adjp": adj_bf,
            "w2t": w2th,
            "idx2": _tile128(idx[c * T:(c + 1) * T]),
            "pw2": _tile128(pw[c * T:(c + 1) * T]),
            "gidx": _tile128(grow[c * T:(c + 1) * T]).astype(np.int32),
            "scidx": np.ascontiguousarray(scidx[c]).astype(np.int32),
        })
    return in_maps, (slot_sizes, Q)


def _run(inputs, trace=False):
    in_maps, (slot_sizes, Q) = _prepare_inputs(
        inputs["x"], inputs["selected_indices"], inputs["pattern_weights"],
        inputs["base_patterns"], inputs["cm_w"], inputs["cm_b"],
        inputs["adj_proj"], inputs["w2_w"])
    nc = _build_program(slot_sizes, Q)
    res = run_bass_kernel_spmd(nc, in_maps, core_ids=list(range(NCORES)), trace=trace)
    out = np.concatenate([res.results[c]["out"] for c in range(NCORES)], axis=0)
    out = out + np.asarray(inputs["w2_b"], dtype=np.float32)[None, :]
    return out.reshape(B, S, D).astype(np.float32), res


def kernel(**inputs) -> np.ndarray:
    out, _ = _run(inputs, trace=False)
    return out


# revision 28
# speedup vs baseline: 1.2343x; 1.2066x over previous
"""ContextualNeuronPool Trainium2 kernel (8-core SPMD).

Math (per token t, with K=8 selected pool entries p_k = idx[t,k], w = softmax(pattern_weights[t])):
    combined[t, f] = sum_k w_k * bp_eff[p_k, f]                  (base term, via routing matrix A)
                   + (sum_k w_k * (G[p_k] @ x[t])) @ adj_proj    (modulation term, via MoE grouping)
    out[t] = gelu(combined[t]) @ W2^T + w2_b
where G[p] = cm_w block [64, 1024] for pool entry p and bp_eff folds the cm_b bias:
    bp_eff = base_patterns + cm_b.reshape(P, M) @ adj_proj       (exact constant folding, host side)

Sharding:
  Phase A (expert-sharded): core c owns pool entries [64c, 64c+64). Host groups (token, k)
  pairs by pool entry. Per core, entries are sorted by token count into 64 slots; slot s is
  padded to a global slot_sizes[s] so all cores run an identical program. Host ships gathered
  x columns (bf16); modulation vectors come from per-slot matmul chains. Each slot's vectors
  are indirect-scattered into a pair table sorted by destination core (pad rows dropped via
  OOB), AllToAll'd (each pair goes only to the core owning its token), then gathered per
  token tile by index.
  Phase B (token-sharded): core c owns tokens [512c, 512c+512). Softmax, routing matrix A,
  base-term matmul A @ bp_eff (overlaps phase A DMA), + modulation @ adj_proj, gelu, @ W2^T.
"""

import collections

import numpy as np
import ml_dtypes

import concourse.bacc as bacc
import concourse.bass as bass
import concourse.tile as tile
import concourse.mybir as mybir
from concourse.bass_utils import run_bass_kernel_spmd
from concourse.masks import make_identity

BF16 = mybir.dt.bfloat16
F32 = mybir.dt.float32
I32 = mybir.dt.int32
AF = mybir.ActivationFunctionType
ALU = mybir.AluOpType

POOL, D, DFF, M = 512, 1024, 4096, 64
B, S, K = 2, 2048, 8
NCORES = 8
NTOK = B * S                  # 4096 tokens
T = NTOK // NCORES            # 512 tokens per core
EPC = POOL // NCORES          # 64 experts (pool entries) per core
DC = D // 128                 # 8 contraction chunks
TT = T // 128                 # 4 token tiles per core
PC = POOL // 128              # 4 pool chunks
FT = DFF // 128               # 32 d_ff tiles
GRP = 8                       # expert slots per DMA load group
NGRP = EPC // GRP             # 8 groups
W2C = 4                       # f-tiles per W2 chunk tile
PAD_IDX = 1 << 20             # scatter index for padding rows (OOB-dropped)


def _build_program(slot_sizes, Q):
    slot_off = np.concatenate([[0], np.cumsum(slot_sizes)]).astype(int)
    TW = int(slot_off[-1])          # packed xgt width
    NA2 = NCORES * Q                # pair table rows (src-chunked)
    NTILE = (TW + 127) // 128       # packed scatter tiles

    nc = bacc.Bacc("TRN2", target_bir_lowering=False, debug=False, num_devices=NCORES)

    xgt_d = nc.dram_tensor("xgt", [128, DC, TW], BF16, kind="ExternalInput")
    cmt_d = nc.dram_tensor("cmt", [128, DC, EPC * M], BF16, kind="ExternalInput")
    bp_d = nc.dram_tensor("bp", [POOL, DFF], BF16, kind="ExternalInput")
    adj_d = nc.dram_tensor("adjp", [M, DFF], BF16, kind="ExternalInput")
    w2t_d = nc.dram_tensor("w2t", [128, FT, D], BF16, kind="ExternalInput")
    idx_d = nc.dram_tensor("idx2", [128, TT * K], I32, kind="ExternalInput")
    pw_d = nc.dram_tensor("pw2", [128, TT * K], F32, kind="ExternalInput")
    gidx_d = nc.dram_tensor("gidx", [128, TT * K], I32, kind="ExternalInput")
    scidx_d = nc.dram_tensor("scidx", [128, NTILE], I32, kind="ExternalInput")
    out_d = nc.dram_tensor("out", [T, D], F32, kind="ExternalOutput")

    with tile.TileContext(nc) as tc:
        with tc.tile_pool(name="const", bufs=1) as const, \
             tc.tile_pool(name="small", bufs=1) as small, \
             tc.tile_pool(name="ab", bufs=2) as ab_pool, \
             tc.tile_pool(name="rg", bufs=8) as rg_pool, \
             tc.tile_pool(name="rw", bufs=2) as rw_pool, \
             tc.tile_pool(name="comb", bufs=3) as comb_pool, \
             tc.tile_pool(name="outp", bufs=2) as out_pool, \
             tc.tile_pool(name="dram", bufs=1, space="DRAM") as dram:

            # ---------------- constants / small inputs ----------------
            ident = const.tile([128, 128], BF16)
            make_identity(nc, ident[:])
            # wideident[:, 128+i] = e_i : shifted-identity bank for the packing
            # matmuls (partition-offset placement without partition-offset APs)
            wideident = const.tile([128, 384], BF16, tag="wident")
            nc.vector.memset(wideident[:], 0.0)
            nc.vector.tensor_copy(out=wideident[:, 128:256], in_=ident[:])
            iota_f = const.tile([128, POOL], F32)
            nc.gpsimd.iota(iota_f[:], pattern=[[1, POOL]], base=0, channel_multiplier=0,
                           allow_small_or_imprecise_dtypes=True)

            idx_i = small.tile([128, TT, K], I32)
            pw_sb = small.tile([128, TT, K], F32)
            gidx_sb = small.tile([128, TT * K], I32)
            scidx_sb = small.tile([128, NTILE], I32)
            nc.scalar.dma_start(out=scidx_sb[:], in_=scidx_d[:, :])
            nc.scalar.dma_start(out=idx_i[:], in_=idx_d[:, :])
            nc.scalar.dma_start(out=pw_sb[:], in_=pw_d[:, :])
            nc.scalar.dma_start(out=gidx_sb[:], in_=gidx_d[:, :])

            idxf = small.tile([128, TT, K], F32)
            w_sb = small.tile([128, TT, K], F32)
            negmax = small.tile([128, TT, 1], F32)
            sume = small.tile([128, TT, 1], F32)
            rec = small.tile([128, TT, 1], F32)
            for ti in range(TT):
                nc.vector.reduce_max(out=negmax[:, ti], in_=pw_sb[:, ti],
                                     axis=mybir.AxisListType.X, negate=True)
                nc.scalar.activation(out=w_sb[:, ti], in_=pw_sb[:, ti], func=AF.Exp,
                                     bias=negmax[:, ti], scale=1.0, accum_out=sume[:, ti])
                nc.vector.reciprocal(out=rec[:, ti], in_=sume[:, ti])
                nc.vector.tensor_scalar_mul(out=w_sb[:, ti], in0=w_sb[:, ti], scalar1=rec[:, ti])
            nc.vector.tensor_copy(out=idxf[:], in_=idx_i[:])

            pair_tab = dram.tile([NA2, M], BF16)
            at2_tab = dram.tile([NA2, M], BF16)

            at_tiles = []   # A^T chunk tiles [128 pool, T] bf16
            for pj in range(PC):
                at_tiles.append(const.tile([128, T], BF16, tag=f"at{pj}", name=f"at{pj}"))
            wmodT = const.tile([M, T], BF16, tag="wmodT")
            stage_tiles = []
            act_tiles = []
            for ft in range(FT):
                stage_tiles.append(const.tile([128, T], BF16, tag=f"stg{ft}", name=f"stg{ft}"))
                act_tiles.append(const.tile([128, T], BF16, tag=f"act{ft}", name=f"act{ft}"))
            adj_sb = const.tile([M, DFF], BF16, tag="adj")

            # helper: A build (DVE) for one token tile
            def a_build(ti):
                a_t = ab_pool.tile([128, POOL], BF16, tag="a", name=f"a{ti}")
                for k in range(K):
                    if k == 0:
                        nc.vector.tensor_scalar(out=a_t[:], in0=iota_f[:],
                                                scalar1=idxf[:, ti, k:k + 1],
                                                scalar2=w_sb[:, ti, k:k + 1],
                                                op0=ALU.is_equal, op1=ALU.mult)
                    else:
                        tmp = ab_pool.tile([128, POOL], BF16, tag="atmp")
                        nc.vector.tensor_scalar(out=tmp[:], in0=iota_f[:],
                                                scalar1=idxf[:, ti, k:k + 1],
                                                scalar2=w_sb[:, ti, k:k + 1],
                                                op0=ALU.is_equal, op1=ALU.mult)
                        nc.vector.tensor_tensor(out=a_t[:], in0=a_t[:], in1=tmp[:], op=ALU.add)
                return a_t

            with tc.tile_pool(name="psA", bufs=2, space="PSUM") as psA, \
                 tc.tile_pool(name="psT", bufs=2, space="PSUM") as psT, \
                 tc.tile_pool(name="psP", bufs=2, space="PSUM") as psP, \
                 tc.tile_pool(name="psB", bufs=2, space="PSUM") as psB, \
                 tc.tile_pool(name="xg", bufs=3) as xg_pool, \
                 tc.tile_pool(name="cm", bufs=3) as cm_pool, \
                 tc.tile_pool(name="pr", bufs=16) as pr_pool, \
                 tc.tile_pool(name="bpp", bufs=1) as bp_pool:

                bp_tiles = []
                for pj in range(PC):
                    t_ = bp_pool.tile([128, DFF], BF16, tag=f"bp{pj}", name=f"bp{pj}")
                    bp_tiles.append(t_)

                # sync queue: group loads g0-g3, bp, g4-g7, adj, w2
                xg_tiles, cm_tiles = {}, {}

                def load_group(g):
                    glo, ghi = int(slot_off[g * GRP]), int(slot_off[(g + 1) * GRP])
                    gw = ghi - glo
                    xg = xg_pool.tile([128, DC, gw], BF16, tag="xg", name=f"xg{g}")
                    cm = cm_pool.tile([128, DC, GRP * M], BF16, tag="cm", name=f"cm{g}")
                    nc.sync.dma_start(out=xg[:], in_=xgt_d[:, :, glo:ghi])
                    nc.sync.dma_start(
                        out=cm[:], in_=cmt_d[:, :, g * GRP * M:(g + 1) * GRP * M])
                    xg_tiles[g], cm_tiles[g] = xg, cm

                # bp/adj on the scalar queue so group loads stream unimpeded
                for g in range(NGRP):
                    load_group(g)
                for pj in range(PC):
                    nc.scalar.dma_start(out=bp_tiles[pj][:], in_=bp_d[pj * 128:(pj + 1) * 128, :])
                nc.scalar.dma_start(out=adj_sb[:], in_=adj_d[:, :])

                # phase A slots: PE chain -> psA -> DVE copy (offset 0) -> sbs;
                # PE "placement matmuls" (shifted identity) accumulate slot rows
                # into packed 128-row PSUM tiles; one gpsimd scatter per tile.
                # Precompute per-tile contribution counts for start/stop flags.
                contribs = collections.Counter()
                for sl in range(EPC):
                    off, m_s = int(slot_off[sl]), int(slot_sizes[sl])
                    contribs[off // 128] += 1
                    if (off % 128) + m_s > 128:
                        contribs[off // 128 + 1] += 1
                seen = collections.Counter()
                psP_tiles = {}

                def flush_tile(tl):
                    pr = pr_pool.tile([128, M], BF16, tag="pr", name=f"pr{tl}")
                    nc.vector.tensor_copy(out=pr[:], in_=psP_tiles[tl][:])
                    nc.gpsimd.indirect_dma_start(
                        out=pair_tab[:],
                        out_offset=bass.IndirectOffsetOnAxis(
                            ap=scidx_sb[:, tl:tl + 1], axis=0),
                        in_=pr[:], in_offset=None)

                def place(tl, sbs, m_s, lhs_cols):
                    if tl not in psP_tiles:
                        psP_tiles[tl] = psP.tile([128, M], F32, tag="pk", name=f"pk{tl}")
                    seen[tl] += 1
                    nc.tensor.matmul(
                        psP_tiles[tl][:],
                        lhsT=wideident[0:m_s, lhs_cols:lhs_cols + 128],
                        rhs=sbs[:m_s, :],
                        start=(seen[tl] == 1), stop=(seen[tl] == contribs[tl]))
                    if seen[tl] == contribs[tl]:
                        flush_tile(tl)

                def slots_group(g):
                    glo = int(slot_off[g * GRP])
                    xg, cm = xg_tiles[g], cm_tiles[g]
                    for s in range(GRP):
                        sl = g * GRP + s
                        m_s = int(slot_sizes[sl])
                        lo = int(slot_off[sl] - glo)
                        ps = psA.tile([128, M], F32)
                        for j in range(DC):
                            nc.tensor.matmul(ps[:m_s, :],
                                             lhsT=xg[:, j, lo:lo + m_s],
                                             rhs=cm[:, j, s * M:(s + 1) * M],
                                             start=(j == 0), stop=(j == DC - 1))
                        sbs = pr_pool.tile([128, M], BF16, tag="sbs", name=f"sbs{sl}")
                        nc.vector.tensor_copy(out=sbs[:m_s, :], in_=ps[:m_s, :])
                        off = int(slot_off[sl])
                        tl, part = off // 128, off % 128
                        n1 = min(m_s, 128 - part)
                        place(tl, sbs, n1, 128 - part)
                        if n1 < m_s:
                            place(tl + 1, sbs, m_s, 128 + n1)

                # A^T transposes for one token tile (PE + scalar copies)
                def at_transpose(ti, a_t):
                    for pj in range(PC):
                        pst = psT.tile([128, 128], BF16)
                        nc.tensor.transpose(pst[:], a_t[:, pj * 128:(pj + 1) * 128], ident[:])
                        nc.scalar.activation(out=at_tiles[pj][:, ti * 128:(ti + 1) * 128],
                                             in_=pst[:], func=AF.Copy)

                # base term: combined^T staged = (A @ bp_eff)^T
                def base_chains(fts):
                    for ft in fts:
                        psb = psB.tile([128, T], F32)
                        for pj in range(PC):
                            nc.tensor.matmul(psb[:],
                                             lhsT=bp_tiles[pj][:, ft * 128:(ft + 1) * 128],
                                             rhs=at_tiles[pj][:],
                                             start=(pj == 0), stop=(pj == PC - 1))
                        nc.scalar.activation(out=stage_tiles[ft][:], in_=psb[:], func=AF.Copy)

                # interleaved emission: phase A groups + A build + transposes + base.
                # base 16-31 sit after the last slot group so the PE stays busy
                # through the AllToAll window.
                for ti in range(TT):
                    a_t = a_build(ti)               # DVE
                    slots_group(ti)                 # PE g0..g3 + DVE pr copies
                    at_transpose(ti, a_t)           # PE (after slots of this group)
                slots_group(4)
                slots_group(5)
                slots_group(6)
                base_chains(range(0, 16))
                slots_group(7)
                base_chains(range(16, FT))

                nc.gpsimd.collective_compute(
                    "AllToAll", ALU.bypass,
                    replica_groups=[list(range(NCORES))],
                    ins=[pair_tab[:].opt()], outs=[at2_tab[:].opt()],
                )

                # all 32 gathers issued up front on the gpsimd queue (serialized
                # ~1.1us each); consumers pipeline per token tile.
                rg_tiles = {}
                for ti in range(TT):
                    for k in range(K):
                        rgt = rg_pool.tile([128, M], BF16, tag="rg", name=f"rg{ti}_{k}")
                        nc.gpsimd.indirect_dma_start(
                            out=rgt[:], out_offset=None,
                            in_=at2_tab[:],
                            in_offset=bass.IndirectOffsetOnAxis(
                                ap=gidx_sb[:, ti * K + k: ti * K + k + 1], axis=0),
                        )
                        rg_tiles[(ti, k)] = rgt

            # ---------------- pass 2: token-half pipelined tail ----------------
            # weighted k-sum via scalar scale + PE transpose-accumulate
            with tc.tile_pool(name="psW", bufs=1, space="PSUM") as psW, \
                 tc.tile_pool(name="w2p", bufs=1) as w2_pool:

                def wmod_ti(ti):
                    rw = rw_pool.tile([128, K, M], BF16, tag="rw")
                    for k in range(K):
                        nc.scalar.activation(out=rw[:, k], in_=rg_tiles[(ti, k)][:],
                                             func=AF.Copy, scale=w_sb[:, ti, k:k + 1])
                    psw = psW.tile([M, 128], F32)
                    for k in range(K):
                        nc.tensor.matmul(psw[:], lhsT=rw[:, k], rhs=ident[:],
                                         start=(k == 0), stop=(k == K - 1))
                    nc.vector.tensor_copy(out=wmodT[:, ti * 128:(ti + 1) * 128], in_=psw[:])

                w2_tiles = []
                for j in range(FT // W2C):
                    w2_tiles.append(w2_pool.tile([128, W2C, D], BF16, tag=f"w2_{j}",
                                                 name=f"w2_{j}"))
                    nc.sync.dma_start(out=w2_tiles[j][:],
                                      in_=w2t_d[:, j * W2C:(j + 1) * W2C, :])

                def adj_half(h, psC):
                    sl = slice(h * 256, (h + 1) * 256)
                    for ft in range(FT):
                        psc = psC.tile([128, 256], F32, tag="psc", name=f"psc{h}_{ft}")
                        nc.tensor.matmul(psc[:], lhsT=adj_sb[:, ft * 128:(ft + 1) * 128],
                                         rhs=wmodT[:, sl], start=True, stop=True)
                        comb = comb_pool.tile([128, 256], BF16, tag="comb",
                                              name=f"comb{h}_{ft}")
                        nc.vector.tensor_tensor(out=comb[:], in0=stage_tiles[ft][:, sl],
                                                in1=psc[:], op=ALU.add)
                        nc.scalar.activation(out=act_tiles[ft][:, sl], in_=comb[:],
                                             func=AF.Gelu)

                def w2_half(psO, trange):
                    for fc in range(FT):
                        w2c = w2_tiles[fc // W2C]
                        for t in trange:
                            for dd in range(2):
                                nc.tensor.matmul(psO[(t, dd)][:],
                                                 lhsT=act_tiles[fc][:, t * 128:(t + 1) * 128],
                                                 rhs=w2c[:, fc % W2C, dd * 512:(dd + 1) * 512],
                                                 start=(fc == 0), stop=(fc == FT - 1))

                def drain_half(psO, trange):
                    for t in trange:
                        for dd in range(2):
                            ob = out_pool.tile([128, 512], F32)
                            nc.vector.tensor_copy(out=ob[:], in_=psO[(t, dd)][:])
                            nc.sync.dma_start(
                                out=out_d[t * 128:(t + 1) * 128, dd * 512:(dd + 1) * 512],
                                in_=ob[:])

                with tc.tile_pool(name="psO", bufs=1, space="PSUM") as psO_pool, \
                     tc.tile_pool(name="psC", bufs=2, space="PSUM") as psC:
                    def alloc_psO(trange, nm):
                        return {(t, dd): psO_pool.tile([128, 512], F32, tag=f"o{t % 2}_{dd}",
                                                       name=f"o{nm}{t}_{dd}")
                                for t in trange for dd in range(2)}
                    wmod_ti(0)
                    wmod_ti(1)
                    adj_half(0, psC)
                    psOa = alloc_psO((0, 1), "a")
                    w2_half(psOa, (0, 1))
                    wmod_ti(2)
                    wmod_ti(3)
                    adj_half(1, psC)
                    drain_half(psOa, (0, 1))
                    psOb = alloc_psO((2, 3), "b")
                    w2_half(psOb, (2, 3))
                    drain_half(psOb, (2, 3))

    nc.compile()
    return nc


def _routing(idx):
    """Group (t, k) pairs by pool entry; build slot packing + AllToAll chunk layout."""
    flat_e = idx.ravel()
    order = np.argsort(flat_e, kind="stable")  # pairs sorted by (expert, t, k)
    counts = np.bincount(flat_e, minlength=POOL)
    starts = np.zeros(POOL, dtype=np.int64)
    starts[1:] = np.cumsum(counts)[:-1]
    tok_sorted = (np.arange(NTOK * K, dtype=np.int64) // K)[order]

    # per core: experts sorted by count desc -> slots
    slot_expert = np.zeros((NCORES, EPC), dtype=np.int64)
    for c in range(NCORES):
        cnt = counts[c * EPC:(c + 1) * EPC]
        slot_expert[c] = c * EPC + np.argsort(-cnt, kind="stable")
    slot_counts = counts[slot_expert]                       # [NCORES, EPC]
    slot_sizes = ((slot_counts.max(axis=0) + 15) // 16 * 16).astype(np.int64)
    slot_sizes = np.maximum(slot_sizes, 16)
    assert slot_sizes.max() <= 128, f"slot overflow {slot_sizes.max()}"
    slot_off = np.concatenate([[0], np.cumsum(slot_sizes)])

    # AllToAll chunk sizes: pairs per (src expert-core, dst token-core)
    src_of_pair = flat_e // EPC
    dst_of_pair = (np.arange(NTOK * K, dtype=np.int64) // K) // T
    cnt_cd = np.bincount(src_of_pair * NCORES + dst_of_pair,
                         minlength=NCORES * NCORES).reshape(NCORES, NCORES)
    Q = int((cnt_cd.max() + 15) // 16 * 16)

    # per-pair table row (in src core's pair_tab, dst-chunked) + receiver row.
    # scidx is packed: column tl covers packed pair positions [tl*128, tl*128+128);
    # padding positions map to in-range rows that hold no real pair (chunk tails)
    # so pad writes land on junk rows the receivers never gather.
    TW = int(slot_off[-1])
    NTILE = (TW + 127) // 128
    if NCORES * Q - NTOK * K // NCORES < 128:
        Q += 16
    NA2 = NCORES * Q
    scidx = np.zeros((NCORES, 128, NTILE), dtype=np.int64)
    grow = np.empty(NTOK * K, dtype=np.int64)   # row in receiver's at2 = src*Q + rank
    for c in range(NCORES):
        rank = np.zeros(NCORES, dtype=np.int64)
        rows = np.zeros((128, NTILE), dtype=np.int64)
        filled = np.zeros((128, NTILE), dtype=bool)
        for s in range(EPC):
            e = int(slot_expert[c, s])
            seg = order[starts[e]:starts[e] + counts[e]]   # pair flat ids, token-sorted
            toks = tok_sorted[starts[e]:starts[e] + counts[e]]
            for i in range(len(seg)):
                d = int(toks[i]) // T
                r = rank[d]; rank[d] += 1
                pp = int(slot_off[s]) + i
                rows[pp % 128, pp // 128] = d * Q + r
                filled[pp % 128, pp // 128] = True
                grow[seg[i]] = c * Q + r
        assert rank.max() <= Q
        # unused (junk) rows: chunk tails
        junk = np.concatenate([np.arange(d * Q + rank[d], (d + 1) * Q)
                               for d in range(NCORES)])
        assert len(junk) >= 128, f"not enough junk rows ({len(junk)})"
        pad_rows = junk[:128]
        pads = ~filled
        rows[pads] = np.broadcast_to(pad_rows[:, None], (128, NTILE))[pads]
        scidx[c] = rows
    grow = grow.reshape(NTOK, K)
    return (order, counts, starts, tok_sorted, slot_expert, slot_sizes, slot_off,
            TW, Q, scidx, grow)


def _tile128(a):
    """[T, X] -> [128, TT*X] with row p holding token ti*128+p at cols ti*X+..."""
    X = a.shape[1]
    return np.ascontiguousarray(
        a.reshape(TT, 128, X).transpose(1, 0, 2).reshape(128, TT * X))


def _prepare_inputs(x, selected_indices, pattern_weights, base_patterns, cm_w, cm_b,
                    adj_proj, w2_w):
    bf = ml_dtypes.bfloat16
    x2 = np.ascontiguousarray(x.reshape(NTOK, D), dtype=np.float32)
    idx = np.ascontiguousarray(selected_indices.reshape(NTOK, K)).astype(np.int32)
    pw = np.ascontiguousarray(pattern_weights.reshape(NTOK, K), dtype=np.float32)

    # exact constant folding of the cm_b bias into the base patterns
    bp_eff = base_patterns.astype(np.float32) + cm_b.reshape(POOL, M).astype(np.float32) @ adj_proj.astype(np.float32)
    bp_bf = bp_eff.astype(bf)
    adj_bf = adj_proj.astype(bf)
    w2t = np.ascontiguousarray(w2_w.T).astype(bf)               # [DFF, D]
    w2th = np.ascontiguousarray(
        w2t.reshape(FT, 128, D).transpose(1, 0, 2))             # [128, FT, D]
    x2t_bf = np.ascontiguousarray(x2.T).astype(bf)              # [D, NTOK]

    (order, counts, starts, tok_sorted, slot_expert, slot_sizes, slot_off, TW,
     Q, scidx, grow) = _routing(idx)

    cm3 = cm_w.reshape(POOL, M, D)
    in_maps = []
    for c in range(NCORES):
        xgt = np.zeros((D, TW), dtype=bf)
        cmt = np.empty((D, EPC * M), dtype=bf)
        for s in range(EPC):
            e = int(slot_expert[c, s])
            seg = tok_sorted[starts[e]:starts[e] + counts[e]]
            off = int(slot_off[s])
            xgt[:, off:off + len(seg)] = x2t_bf[:, seg]
            cmt[:, s * M:(s + 1) * M] = cm3[e].T.astype(bf)
        xgt128 = np.ascontiguousarray(xgt.reshape(DC, 128, TW).transpose(1, 0, 2))
        cmt128 = np.ascontiguousarray(cmt.reshape(DC, 128, EPC * M).transpose(1, 0, 2))
        in_maps.append({
            "xgt": xgt128,
            "cmt": cmt128,
            "bp": bp_bf,
            "adjp": adj_bf,
            "w2t": w2th,
            "idx2": _tile128(idx[c * T:(c + 1) * T]),
            "pw2": _tile128(pw[c * T:(c + 1) * T]),
            "gidx": _tile128(grow[c * T:(c + 1) * T]).astype(np.int32),
            "scidx": np.ascontiguousarray(scidx[c]).astype(np.int32),
        })
    return in_maps, (slot_sizes, Q)


def _run(inputs, trace=False):
    in_maps, (slot_sizes, Q) = _prepare_inputs(
        inputs["x"], inputs["selected_indices"], inputs["pattern_weights"],
        inputs["base_patterns"], inputs["cm_w"], inputs["cm_b"],
        inputs["adj_proj"], inputs["w2_w"])
    nc = _build_program(slot_sizes, Q)
    res = run_bass_kernel_spmd(nc, in_maps, core_ids=list(range(NCORES)), trace=trace)
    out = np.concatenate([res.results[c]["out"] for c in range(NCORES)], axis=0)
    out = out + np.asarray(inputs["w2_b"], dtype=np.float32)[None, :]
    return out.reshape(B, S, D).astype(np.float32), res


def kernel(**inputs) -> np.ndarray:
    out, _ = _run(inputs, trace=False)
    return out
